# revision 14
# baseline (speedup 1.0000x reference)
"""Trainium2 Bass kernel for GQA attention (B=2, S=2048, D=2048, H=32, KV=8, HD=64).

Sharding over 8 NeuronCores: batch (2) x 4-way head tensor-parallel.
Core c handles batch c//4 and KV heads {2r, 2r+1} (r = c%4) with their
8 query heads. After attention, 4-core AllGathers (one per head-pair
half) assemble the full attention output (transposed layout) and each
core computes a 512-column shard of the final wo projection.

All matmuls run in bf16 (inputs converted host-side), accumulation fp32.

Layout tricks (host-side permutations, cancel out in the math):
- wq/wk columns are permuted inside each head's 64-dim block so rope pairs
  (even, odd) become (first-32, last-32) contiguous partition blocks.
- wq columns are ordered so QT tile t holds query head (g0, rep t) in
  partitions 0-63 and (g1, rep t) in partitions 64-127, which lets the
  scores matmuls for the two heads pack into disjoint PE row groups.
- wo rows are permuted to match the AllGather'ed attention-transposed
  row order.

Pipeline structure:
- scores for 2 chunks x 2 heads land in one 4-bank PSUM tile, so each
  exp ACTIVATE covers 2048 elems/partition (amortizes the ~352-cycle
  per-op overhead; ACT then runs faster than the PE and hides).
- scores(s) / exp(s) / PV(s-1) software pipeline keeps the PE from
  stalling on the single-buffered scores PSUM.
- softmax denominators come free via a ones-column appended to V;
  normalization (copy/recip/broadcast/mult) is deferred off the
  critical path (double-buffered PV accumulators).
"""

import numpy as np
import ml_dtypes

import concourse.bass as bass
import concourse.mybir as mybir
import concourse.tile as tile
from concourse import bacc
from concourse.bass_utils import run_bass_kernel_spmd

B, S, D = 2, 2048, 2048
H, KV, HD = 32, 8, 64
NREP = H // KV
P = 128
NCORES = 8
GRP = 4                  # cores per batch group
QCOLS = 8 * HD           # 512 query cols per core
KCOLS = 2 * HD           # 128 k/v cols per core
OCOLS = D // GRP         # 512 output cols per core
DCH = D // P             # 16 contraction chunks
NJQ = S // 512           # 4 q windows
NPAIR = 4                # head pairs per core (one per QT tile)

bf16 = mybir.dt.bfloat16
f32 = mybir.dt.float32
MULT = mybir.AluOpType.mult
ADD = mybir.AluOpType.add
EXP = mybir.ActivationFunctionType.Exp

_BF = ml_dtypes.bfloat16


def _rope(nc, rp, dst, ps, cosw, sinw):
    """dst = ps * cos + swap32(ps) * sin  (rope in pair-split layout)."""
    n = ps.shape[-1]
    ra = rp.tile([P, n], f32, tag="ra", name="ra", bufs=2)
    rb = rp.tile([P, n], f32, tag="rb", name="rb", bufs=2)
    nc.vector.tensor_tensor(out=ra[:], in0=ps[:], in1=cosw, op=MULT)
    for ob, ib in ((0, 32), (32, 0), (64, 96), (96, 64)):
        nc.vector.tensor_tensor(
            out=rb[ob : ob + 32, :],
            in0=ps[ib : ib + 32, :],
            in1=sinw[ob : ob + 32, :],
            op=MULT,
        )
    nc.vector.tensor_tensor(out=dst, in0=ra[:], in1=rb[:], op=ADD)


def build_graph():
    nc = bacc.Bacc("TRN2", target_bir_lowering=False, debug=False, num_devices=NCORES)

    xT = nc.dram_tensor("xT", [D, S], bf16, kind="ExternalInput")
    wq = nc.dram_tensor("wq", [D, QCOLS], bf16, kind="ExternalInput")
    wk = nc.dram_tensor("wk", [D, KCOLS], bf16, kind="ExternalInput")
    wv = nc.dram_tensor("wv", [D, KCOLS], bf16, kind="ExternalInput")
    wo = nc.dram_tensor("wo", [H * HD, OCOLS], bf16, kind="ExternalInput")
    cos4 = nc.dram_tensor("cos4", [P, S], f32, kind="ExternalInput")
    sin4 = nc.dram_tensor("sin4", [P, S], f32, kind="ExternalInput")
    cmask = nc.dram_tensor("cmask", [4 * P, 512], bf16, kind="ExternalInput")
    out = nc.dram_tensor("out", [S, OCOLS], f32, kind="ExternalOutput")

    with tile.TileContext(nc) as tc:
        _build_body(tc, nc, xT, wq, wk, wv, wo, cos4, sin4, cmask, out)
    nc.compile()
    return nc


def _build_body(tc, nc, xT, wq, wk, wv, wo, cos4, sin4, cmask, out):
    from contextlib import ExitStack

    with ExitStack() as ctx:
        const = ctx.enter_context(tc.tile_pool(name="const", bufs=1))
        dram = ctx.enter_context(tc.tile_pool(name="dram", bufs=1, space="DRAM"))

        # weights on the gpsimd DMA queue so the sync queue starts on xT
        # immediately (DMA issue is ~0.6us each and serializes per queue)
        wk_sb = const.tile([P, DCH, KCOLS], bf16)
        wv_sb = const.tile([P, DCH, KCOLS], bf16)
        for c in range(DCH):
            nc.gpsimd.dma_start(wk_sb[:, c, :], wk.ap()[c * P : (c + 1) * P, :])
        for c in range(DCH):
            nc.gpsimd.dma_start(wv_sb[:, c, :], wv.ap()[c * P : (c + 1) * P, :])
        mask_sb = const.tile([P, 4, 512], bf16)

        # long-lived activation tensors
        QT = [const.tile([P, S], bf16, name=f"qt{t}") for t in range(NPAIR)]
        KT = const.tile([P, S], bf16, name="kt")
        V = const.tile([P, DCH, 130], bf16, name="vsb")  # [g0 64 | 1 | g1 64 | 1]
        attT = [const.tile([P, S], bf16, name=f"attT{t}") for t in range(NPAIR)]

        nc.vector.memset(V[:, :, 64], 1.0)
        nc.vector.memset(V[:, :, 129], 1.0)

        # ---- phase 1: projections + rope ----------------------------
        # body-wide PSUM pool: 4 banks (pj0-3). Projections use them as the
        # 4 concurrent js accumulators; attention reuses them (by jq parity)
        # for the PV accumulators, leaving banks 4-7 to the scores pool.
        pps = ctx.enter_context(tc.tile_pool(name="pps", bufs=1, space="PSUM"))

        with tc.tile_pool(name="proj", bufs=1) as proj:
            xt = []
            for c in range(DCH):
                t_ = proj.tile([P, S], bf16, name=f"x{c}", tag=f"x{c}")
                nc.sync.dma_start(t_[:], xT.ap()[c * P : (c + 1) * P, :])
                xt.append(t_)
            nc.sync.dma_start(
                mask_sb[:], cmask.ap().rearrange("(d p) q -> p d q", p=P)
            )
            cos_sb = proj.tile([P, S], f32)
            nc.gpsimd.dma_start(cos_sb[:], cos4.ap())
            sin_sb = proj.tile([P, S], f32)
            nc.gpsimd.dma_start(sin_sb[:], sin4.ap())
            # K projection + rope (weight-chunk outer for LDWEIGHTS reuse)
            kps = [pps.tile([P, 512], f32, tag=f"pj{js}", name="kps", bufs=1)
                   for js in range(NJQ)]
            for c in range(DCH):
                for js in range(NJQ):
                    nc.tensor.matmul(
                        kps[js][:], wk_sb[:, c, :],
                        xt[c][:, js * 512 : (js + 1) * 512],
                        start=(c == 0), stop=(c == DCH - 1),
                    )
            for js in range(NJQ):
                sw = slice(js * 512, (js + 1) * 512)
                _rope(nc, proj, KT[:, sw], kps[js], cos_sb[:, sw], sin_sb[:, sw])
            # V projection (natural layout, seq on partitions)
            for it in range(DCH):
                vp = pps.tile([P, P], f32, tag=f"pj{it % 2}", name="vps", bufs=1)
                for c in range(DCH):
                    nc.tensor.matmul(
                        vp[:], xt[c][:, it * P : (it + 1) * P], wv_sb[:, c, :],
                        start=(c == 0), stop=(c == DCH - 1),
                    )
                nc.vector.tensor_copy(out=V[:, it, 0:64], in_=vp[:, 0:64])
                nc.vector.tensor_copy(out=V[:, it, 65:129], in_=vp[:, 64:128])
            # Q projection + rope (weight-chunk outer)
            wq_sb = const.tile([P, DCH, QCOLS], bf16)
            for c in range(DCH):
                nc.gpsimd.dma_start(wq_sb[:, c, :], wq.ap()[c * P : (c + 1) * P, :])
            for ot in range(NPAIR):
                qps = [pps.tile([P, 512], f32, tag=f"pj{js}", name="qps", bufs=1)
                       for js in range(NJQ)]
                for c in range(DCH):
                    for js in range(NJQ):
                        nc.tensor.matmul(
                            qps[js][:], wq_sb[:, c, ot * P : (ot + 1) * P],
                            xt[c][:, js * 512 : (js + 1) * 512],
                            start=(c == 0), stop=(c == DCH - 1),
                        )
                for js in range(NJQ):
                    sw = slice(js * 512, (js + 1) * 512)
                    _rope(nc, proj, QT[ot][:, sw], qps[js], cos_sb[:, sw],
                          sin_sb[:, sw])

        # ---- phase 2: attention (+ per half-pair AllGather) ----------
        # att_loc/att_all per (pair, half): half h covers q cols [1024h, 1024h+1024)
        att_loc = [[dram.tile([P, 1024], bf16, name=f"aloc{t}_{h}") for h in range(2)]
                   for t in range(NPAIR)]
        att_all = [[dram.tile([GRP * P, 1024], bf16, name=f"aall{t}_{h}") for h in range(2)]
                   for t in range(NPAIR)]

        # wo-phase SBUF opens BEFORE the attention pools so its address range
        # WARs against the (closed) projection pool, not attention tiles --
        # chunk DMAs can then land as soon as each AllGather finishes.
        wos = ctx.enter_context(tc.tile_pool(name="wos", bufs=1))
        wo_sb = wos.tile([P, DCH, OCOLS], bf16)
        for c in range(DCH):
            nc.gpsimd.dma_start(wo_sb[:, c, :], wo.ap()[c * P : (c + 1) * P, :])
        chunks = [[None] * 2 for _ in range(16)]
        for h in range(2):
            for t in range(NPAIR):
                for rr in range(GRP):
                    chunks[4 * t + rr][h] = wos.tile(
                        [P, 1024], bf16, name=f"ach{t}_{rr}_{h}",
                        tag=f"ach{t}_{rr}_{h}",
                    )

        with (
            tc.tile_pool(name="scps", bufs=1, space="PSUM") as scp,
            tc.tile_pool(name="ex", bufs=1) as ex,
            tc.tile_pool(name="nrm", bufs=1) as nrm,
        ):
            def emit_norm(pair, jq, otA, otB):
                qw = slice(jq * 512, (jq + 1) * 512)
                denA = nrm.tile([1, 512], f32, tag="denA", name="denA", bufs=2)
                nc.vector.tensor_copy(out=denA[:], in_=otA[64:65, :])
                denB = nrm.tile([1, 512], f32, tag="denB", name="denB", bufs=2)
                nc.vector.tensor_copy(out=denB[:], in_=otB[64:65, :])
                recA = nrm.tile([1, 512], f32, tag="recA", name="recA", bufs=2)
                nc.vector.reciprocal_approx_fast(out=recA[:], in_=denA[:])
                recB = nrm.tile([1, 512], f32, tag="recB", name="recB", bufs=2)
                nc.vector.reciprocal_approx_fast(out=recB[:], in_=denB[:])
                bcA = nrm.tile([64, 512], f32, tag="bcA", name="bcA", bufs=2)
                nc.gpsimd.partition_broadcast(bcA[:], recA[:])
                bcB = nrm.tile([64, 512], f32, tag="bcB", name="bcB", bufs=2)
                nc.gpsimd.partition_broadcast(bcB[:], recB[:])
                nc.vector.tensor_tensor(
                    out=attT[pair][0:64, qw], in0=otA[0:64, :], in1=bcA[:], op=MULT,
                )
                nc.vector.tensor_tensor(
                    out=attT[pair][64:128, qw], in0=otB[0:64, :], in1=bcB[:],
                    op=MULT,
                )
                if jq % 2 == 1:  # half complete -> ship + gather + preload
                    h = jq // 2
                    hw_ = slice(h * 1024, (h + 1) * 1024)
                    nc.sync.dma_start(att_loc[pair][h][:], attT[pair][:, hw_])
                    nc.gpsimd.collective_compute(
                        "AllGather",
                        mybir.AluOpType.bypass,
                        replica_groups=[[0, 1, 2, 3], [4, 5, 6, 7]],
                        ins=[att_loc[pair][h][:].opt()],
                        outs=[att_all[pair][h][:].opt()],
                    )
                    for rr in range(GRP):
                        nc.sync.dma_start(
                            chunks[4 * pair + rr][h][:],
                            att_all[pair][h][rr * P : (rr + 1) * P, :],
                        )

            def emit_pv(ent):
                eAB2_p, base, otA, otB, nch, pair, jq, W = ent
                for j in (0, 1):
                    ik = base + j
                    first = ik == 0
                    last = ik == nch - 1
                    nc.tensor.matmul(
                        otA[0:65, W:512], V[:, ik, 0:65],
                        eAB2_p[:, 2 * j + 0, W:512],
                        start=first, stop=last,
                    )
                    nc.tensor.matmul(
                        otB[0:65, W:512], V[:, ik, 65:130],
                        eAB2_p[:, 2 * j + 1, W:512],
                        start=first, stop=last,
                    )
                if base + 2 >= nch:  # last chunks of this (pair, jq)
                    emit_norm(pair, jq, otA, otB)

            pend = []  # global software pipeline: scores/exp run 1 ahead of PV
            for pair in range(NPAIR):
                for jq in range(NJQ):
                    qw = slice(jq * 512, (jq + 1) * 512)
                    otA = pps.tile([P, 512], f32, tag=f"pj{2 * (jq % 2)}",
                                   name="otA", bufs=1)
                    otB = pps.tile([P, 512], f32, tag=f"pj{2 * (jq % 2) + 1}",
                                   name="otB", bufs=1)
                    nch = 4 * jq + 4
                    for sc in range(nch // 2):
                        d0 = 2 * sc - 4 * jq
                        W = 256 if d0 == 2 else 0  # causal q-window
                        qwW = slice(jq * 512 + W, (jq + 1) * 512)
                        sAB2 = scp.tile([P, 4, 512], f32, tag="sAB", name="sAB", bufs=1)
                        for j in (0, 1):
                            ik = 2 * sc + j
                            kt_ = slice(ik * P, (ik + 1) * P)
                            nc.tensor.matmul(
                                sAB2[:, 2 * j + 0, W:512], KT[0:64, kt_],
                                QT[pair][0:64, qwW], start=True, stop=True,
                            )
                            nc.tensor.matmul(
                                sAB2[:, 2 * j + 1, W:512], KT[64:128, kt_],
                                QT[pair][64:128, qwW], start=True, stop=True,
                            )
                        eAB2 = ex.tile([P, 4, 512], bf16, tag="eAB", name="eAB", bufs=3)
                        nc.scalar.activation(
                            eAB2[:, :, W:512], sAB2[:, :, W:512], EXP, scale=0.125
                        )
                        for j in (0, 1):
                            d = 2 * sc + j - 4 * jq
                            if d >= 0:  # diagonal chunk: causal mask (both heads)
                                nc.vector.tensor_tensor(
                                    out=eAB2[:, 2 * j : 2 * j + 2, W:512],
                                    in0=eAB2[:, 2 * j : 2 * j + 2, W:512],
                                    in1=mask_sb[:, d : d + 1, W:512].to_broadcast(
                                        (P, 2, 512 - W)
                                    ),
                                    op=MULT,
                                )
                        pend.append((eAB2, 2 * sc, otA, otB, nch, pair, jq, W))
                        if len(pend) > 1:
                            emit_pv(pend.pop(0))
            while pend:
                emit_pv(pend.pop(0))

        # ---- phase 3: wo projection (lo/hi halves overlap last AGs) --
        with tc.tile_pool(name="ops", bufs=3, space="PSUM") as ops:
            for h in range(2):
                for mm in range(8):
                    m = h * 8 + mm
                    mps = ops.tile([P, OCOLS], f32, tag="mps", name="mps", bufs=3)
                    for c2 in range(16):
                        nc.tensor.matmul(
                            mps[:],
                            chunks[c2][h][:, mm * P : (mm + 1) * P],
                            wo_sb[:, c2, :],
                            start=(c2 == 0),
                            stop=(c2 == 15),
                        )
                    osb = wos.tile([P, OCOLS], f32, tag="osb", name="osb", bufs=3)
                    nc.vector.tensor_copy(out=osb[:], in_=mps[:])
                    nc.sync.dma_start(out.ap()[m * P : (m + 1) * P, :], osb[:])


# ---------------------------------------------------------------------------
# host side
# ---------------------------------------------------------------------------

_PERM64 = np.concatenate([np.arange(0, 64, 2), np.arange(1, 64, 2)])


def _qcols(r):
    cols = []
    for t in range(NREP):
        for half in range(2):
            h = (2 * r + half) * NREP + t
            cols.extend(64 * h + _PERM64)
    return np.array(cols)


def _kcols(r):
    cols = []
    for half in range(2):
        g = 2 * r + half
        cols.extend(64 * g + _PERM64)
    return np.array(cols)


def _worows():
    rows = []
    for t in range(NREP):
        for rr in range(GRP):
            for half in range(2):
                h = (2 * rr + half) * NREP + t
                rows.extend(64 * h + np.arange(64))
    return np.array(rows)


def make_in_maps(x, wq, wk, wv, wo, freqs_cos, freqs_sin):
    cosT = np.ascontiguousarray(freqs_cos.T).astype(np.float32)  # (32, S)
    sinT = np.ascontiguousarray(freqs_sin.T).astype(np.float32)
    cos4 = np.ascontiguousarray(np.tile(cosT, (4, 1)))           # (128, S)
    sin4 = np.ascontiguousarray(
        np.concatenate([-sinT, sinT, -sinT, sinT], axis=0)
    )
    cmask = np.triu(np.ones((512, 512), dtype=np.float32)).astype(_BF)

    xT = [np.ascontiguousarray(x[b].T).astype(_BF) for b in range(B)]
    wo_perm = wo[_worows(), :]

    in_maps = []
    for c in range(NCORES):
        b, r = c // GRP, c % GRP
        in_maps.append(
            {
                "xT": xT[b],
                "wq": np.ascontiguousarray(wq[:, _qcols(r)]).astype(_BF),
                "wk": np.ascontiguousarray(wk[:, _kcols(r)]).astype(_BF),
                "wv": np.ascontiguousarray(wv[:, 128 * r : 128 * (r + 1)]).astype(_BF),
                "wo": np.ascontiguousarray(
                    wo_perm[:, OCOLS * r : OCOLS * (r + 1)]
                ).astype(_BF),
                "cos4": cos4,
                "sin4": sin4,
                "cmask": cmask,
            }
        )
    return in_maps


_NC_CACHE = None


def _get_nc():
    global _NC_CACHE
    if _NC_CACHE is None:
        _NC_CACHE = build_graph()
    return _NC_CACHE


def kernel(x, wq, wk, wv, wo, freqs_cos, freqs_sin):
    x = np.asarray(x)
    wq = np.asarray(wq)
    wk = np.asarray(wk)
    wv = np.asarray(wv)
    wo = np.asarray(wo)
    freqs_cos = np.asarray(freqs_cos)
    freqs_sin = np.asarray(freqs_sin)

    in_maps = make_in_maps(x, wq, wk, wv, wo, freqs_cos, freqs_sin)
    nc = _get_nc()
    res = run_bass_kernel_spmd(nc, in_maps, core_ids=list(range(NCORES)))

    out = np.empty((B, S, D), dtype=np.float32)
    for c in range(NCORES):
        b, r = c // GRP, c % GRP
        out[b, :, OCOLS * r : OCOLS * (r + 1)] = res.results[c]["out"]
    return out


# revision 16
# speedup vs baseline: 1.1662x; 1.1662x over previous
"""Trainium2 Bass kernel for GQA attention (B=2, S=2048, D=2048, H=32, KV=8, HD=64).

Sharding over 8 NeuronCores: batch (2) x 4-way head tensor-parallel.
Core c handles batch c//4 and KV heads {2r, 2r+1} (r = c%4) with their
8 query heads. After attention, 4-core AllGathers (one per head-pair
half) assemble the full attention output (transposed layout) and each
core computes a 512-column shard of the final wo projection.

All matmuls run in bf16 (inputs converted host-side), accumulation fp32.

Layout tricks (host-side permutations, cancel out in the math):
- wq/wk columns are permuted inside each head's 64-dim block so rope pairs
  (even, odd) become (first-32, last-32) contiguous partition blocks.
- wq columns are ordered so QT tile t holds query head (g0, rep t) in
  partitions 0-63 and (g1, rep t) in partitions 64-127, which lets the
  scores matmuls for the two heads pack into disjoint PE row groups.
- wo rows are permuted to match the AllGather'ed attention-transposed
  row order.

Pipeline structure:
- scores for 2 chunks x 2 heads land in one 4-bank PSUM tile, so each
  exp ACTIVATE covers 2048 elems/partition (amortizes the ~352-cycle
  per-op overhead; ACT then runs faster than the PE and hides).
- scores(s) / exp(s) / PV(s-1) software pipeline keeps the PE from
  stalling on the single-buffered scores PSUM.
- softmax denominators come free via a ones-column appended to V;
  normalization (copy/recip/broadcast/mult) is deferred off the
  critical path (double-buffered PV accumulators).
"""

import numpy as np
import ml_dtypes

import concourse.bass as bass
import concourse.mybir as mybir
import concourse.tile as tile
from concourse import bacc
from concourse.bass_utils import run_bass_kernel_spmd

B, S, D = 2, 2048, 2048
H, KV, HD = 32, 8, 64
NREP = H // KV
P = 128
NCORES = 8
GRP = 4                  # cores per batch group
QCOLS = 8 * HD           # 512 query cols per core
KCOLS = 2 * HD           # 128 k/v cols per core
OCOLS = D // GRP         # 512 output cols per core
DCH = D // P             # 16 contraction chunks
NJQ = S // 512           # 4 q windows
NPAIR = 4                # head pairs per core (one per QT tile)

bf16 = mybir.dt.bfloat16
f32 = mybir.dt.float32
MULT = mybir.AluOpType.mult
ADD = mybir.AluOpType.add
EXP = mybir.ActivationFunctionType.Exp

_BF = ml_dtypes.bfloat16


def _rope(nc, rp, dst, ps, cosw, sinw):
    """dst = ps * cos + swap32(ps) * sin  (rope in pair-split layout)."""
    n = ps.shape[-1]
    ra = rp.tile([P, n], f32, tag="ra", name="ra", bufs=2)
    rb = rp.tile([P, n], f32, tag="rb", name="rb", bufs=2)
    nc.vector.tensor_tensor(out=ra[:], in0=ps[:], in1=cosw, op=MULT)
    for ob, ib in ((0, 32), (32, 0), (64, 96), (96, 64)):
        nc.vector.tensor_tensor(
            out=rb[ob : ob + 32, :],
            in0=ps[ib : ib + 32, :],
            in1=sinw[ob : ob + 32, :],
            op=MULT,
        )
    nc.vector.tensor_tensor(out=dst, in0=ra[:], in1=rb[:], op=ADD)


def build_graph():
    nc = bacc.Bacc("TRN2", target_bir_lowering=False, debug=False, num_devices=NCORES)

    xT = nc.dram_tensor("xT", [D, S], bf16, kind="ExternalInput")
    wq = nc.dram_tensor("wq", [D, QCOLS], bf16, kind="ExternalInput")
    wk = nc.dram_tensor("wk", [D, KCOLS], bf16, kind="ExternalInput")
    wv = nc.dram_tensor("wv", [D, KCOLS], bf16, kind="ExternalInput")
    wo = nc.dram_tensor("wo", [H * HD, OCOLS], bf16, kind="ExternalInput")
    cos4 = nc.dram_tensor("cos4", [P, S], f32, kind="ExternalInput")
    sin4 = nc.dram_tensor("sin4", [P, S], f32, kind="ExternalInput")
    cmask = nc.dram_tensor("cmask", [4 * P, 512], bf16, kind="ExternalInput")
    out = nc.dram_tensor("out", [S, OCOLS], f32, kind="ExternalOutput")

    with tile.TileContext(nc) as tc:
        _build_body(tc, nc, xT, wq, wk, wv, wo, cos4, sin4, cmask, out)
    nc.compile()
    return nc


def _build_body(tc, nc, xT, wq, wk, wv, wo, cos4, sin4, cmask, out):
    from contextlib import ExitStack

    with ExitStack() as ctx:
        const = ctx.enter_context(tc.tile_pool(name="const", bufs=1))
        dram = ctx.enter_context(tc.tile_pool(name="dram", bufs=1, space="DRAM"))

        # weights on the gpsimd DMA queue so the sync queue starts on xT
        # immediately (DMA issue is ~0.6us each and serializes per queue)
        wk_sb = const.tile([P, DCH, KCOLS], bf16)
        wv_sb = const.tile([P, DCH, KCOLS], bf16)
        for c in range(DCH):
            nc.gpsimd.dma_start(wk_sb[:, c, :], wk.ap()[c * P : (c + 1) * P, :])
        for c in range(DCH):
            nc.gpsimd.dma_start(wv_sb[:, c, :], wv.ap()[c * P : (c + 1) * P, :])
        mask_sb = const.tile([P, 4, 512], bf16)
        nc.gpsimd.dma_start(mask_sb[:], cmask.ap().rearrange("(d p) q -> p d q", p=P))

        # long-lived activation tensors
        QT = [const.tile([P, S], bf16, name=f"qt{t}") for t in range(NPAIR)]
        KT = const.tile([P, S], bf16, name="kt")
        V = const.tile([P, DCH, 130], bf16, name="vsb")  # [g0 64 | 1 | g1 64 | 1]
        attT = [const.tile([P, S], bf16, name=f"attT{t}") for t in range(NPAIR)]

        nc.vector.memset(V[:, :, 64], 1.0)
        nc.vector.memset(V[:, :, 129], 1.0)

        # ---- phase 1: projections + rope ----------------------------
        with (
            tc.tile_pool(name="proj", bufs=1) as proj,
            tc.tile_pool(name="pps", bufs=1, space="PSUM") as pps,
        ):
            xt = []
            for c in range(DCH):
                t_ = proj.tile([P, S], bf16, name=f"x{c}", tag=f"x{c}")
                nc.sync.dma_start(t_[:], xT.ap()[c * P : (c + 1) * P, :])
                xt.append(t_)
            cos_sb = proj.tile([P, S], f32)
            nc.gpsimd.dma_start(cos_sb[:], cos4.ap())
            sin_sb = proj.tile([P, S], f32)
            nc.gpsimd.dma_start(sin_sb[:], sin4.ap())
            # K projection + rope (weight-chunk outer for LDWEIGHTS reuse)
            kps = [pps.tile([P, 512], f32, tag=f"pj{js}", name="kps", bufs=2)
                   for js in range(NJQ)]
            for c in range(DCH):
                for js in range(NJQ):
                    nc.tensor.matmul(
                        kps[js][:], wk_sb[:, c, :],
                        xt[c][:, js * 512 : (js + 1) * 512],
                        start=(c == 0), stop=(c == DCH - 1),
                    )
            for js in range(NJQ):
                sw = slice(js * 512, (js + 1) * 512)
                _rope(nc, proj, KT[:, sw], kps[js], cos_sb[:, sw], sin_sb[:, sw])
            # V projection (natural layout, seq on partitions)
            for it in range(DCH):
                vp = pps.tile([P, P], f32, tag="pj0", name="vps", bufs=2)
                for c in range(DCH):
                    nc.tensor.matmul(
                        vp[:], xt[c][:, it * P : (it + 1) * P], wv_sb[:, c, :],
                        start=(c == 0), stop=(c == DCH - 1),
                    )
                nc.vector.tensor_copy(out=V[:, it, 0:64], in_=vp[:, 0:64])
                nc.vector.tensor_copy(out=V[:, it, 65:129], in_=vp[:, 64:128])
            # Q projection + rope (weight-chunk outer)
            wq_sb = const.tile([P, DCH, QCOLS], bf16)
            for c in range(DCH):
                nc.gpsimd.dma_start(wq_sb[:, c, :], wq.ap()[c * P : (c + 1) * P, :])
            for ot in range(NPAIR):
                qps = [pps.tile([P, 512], f32, tag=f"pj{js}", name="qps", bufs=2)
                       for js in range(NJQ)]
                for c in range(DCH):
                    for js in range(NJQ):
                        nc.tensor.matmul(
                            qps[js][:], wq_sb[:, c, ot * P : (ot + 1) * P],
                            xt[c][:, js * 512 : (js + 1) * 512],
                            start=(c == 0), stop=(c == DCH - 1),
                        )
                for js in range(NJQ):
                    sw = slice(js * 512, (js + 1) * 512)
                    _rope(nc, proj, QT[ot][:, sw], qps[js], cos_sb[:, sw],
                          sin_sb[:, sw])

        # ---- phase 2: attention (+ per half-pair AllGather) ----------
        # att_loc/att_all per (pair, half): half h covers q cols [1024h, 1024h+1024)
        att_loc = [[dram.tile([P, 1024], bf16, name=f"aloc{t}_{h}") for h in range(2)]
                   for t in range(NPAIR)]
        att_all = [[dram.tile([GRP * P, 1024], bf16, name=f"aall{t}_{h}") for h in range(2)]
                   for t in range(NPAIR)]

        # wo-phase SBUF opens BEFORE the attention pools so its address range
        # WARs against the (closed) projection pool, not attention tiles --
        # chunk DMAs can then land as soon as each AllGather finishes.
        wos = ctx.enter_context(tc.tile_pool(name="wos", bufs=1))
        wo_sb = wos.tile([P, DCH, OCOLS], bf16)
        for c in range(DCH):
            nc.gpsimd.dma_start(wo_sb[:, c, :], wo.ap()[c * P : (c + 1) * P, :])
        chunks = [[None] * 2 for _ in range(16)]
        for h in range(2):
            for t in range(NPAIR):
                for rr in range(GRP):
                    chunks[4 * t + rr][h] = wos.tile(
                        [P, 1024], bf16, name=f"ach{t}_{rr}_{h}",
                        tag=f"ach{t}_{rr}_{h}",
                    )

        with (
            tc.tile_pool(name="scps", bufs=1, space="PSUM") as scp,
            tc.tile_pool(name="otps", bufs=1, space="PSUM") as otp,
            tc.tile_pool(name="ex", bufs=1) as ex,
            tc.tile_pool(name="nrm", bufs=1) as nrm,
        ):
            def emit_norm(pair, jq, otA, otB):
                qw = slice(jq * 512, (jq + 1) * 512)
                denA = nrm.tile([1, 512], f32, tag="denA", name="denA", bufs=2)
                nc.vector.tensor_copy(out=denA[:], in_=otA[64:65, :])
                denB = nrm.tile([1, 512], f32, tag="denB", name="denB", bufs=2)
                nc.vector.tensor_copy(out=denB[:], in_=otB[64:65, :])
                recA = nrm.tile([1, 512], f32, tag="recA", name="recA", bufs=2)
                nc.vector.reciprocal_approx_fast(out=recA[:], in_=denA[:])
                recB = nrm.tile([1, 512], f32, tag="recB", name="recB", bufs=2)
                nc.vector.reciprocal_approx_fast(out=recB[:], in_=denB[:])
                bcA = nrm.tile([64, 512], f32, tag="bcA", name="bcA", bufs=2)
                nc.gpsimd.partition_broadcast(bcA[:], recA[:])
                bcB = nrm.tile([64, 512], f32, tag="bcB", name="bcB", bufs=2)
                nc.gpsimd.partition_broadcast(bcB[:], recB[:])
                nc.vector.tensor_tensor(
                    out=attT[pair][0:64, qw], in0=otA[0:64, :], in1=bcA[:], op=MULT,
                )
                nc.vector.tensor_tensor(
                    out=attT[pair][64:128, qw], in0=otB[0:64, :], in1=bcB[:],
                    op=MULT,
                )
                if jq % 2 == 1:  # half complete -> ship + gather + preload
                    h = jq // 2
                    hw_ = slice(h * 1024, (h + 1) * 1024)
                    nc.sync.dma_start(att_loc[pair][h][:], attT[pair][:, hw_])
                    nc.gpsimd.collective_compute(
                        "AllGather",
                        mybir.AluOpType.bypass,
                        replica_groups=[[0, 1, 2, 3], [4, 5, 6, 7]],
                        ins=[att_loc[pair][h][:].opt()],
                        outs=[att_all[pair][h][:].opt()],
                    )
                    for rr in range(GRP):
                        nc.sync.dma_start(
                            chunks[4 * pair + rr][h][:],
                            att_all[pair][h][rr * P : (rr + 1) * P, :],
                        )

            def emit_pv(ent):
                eAB2_p, base, otA, otB, nch, pair, jq = ent
                for j in (0, 1):
                    ik = base + j
                    first = ik == 0
                    last = ik == nch - 1
                    nc.tensor.matmul(
                        otA[0:65, :], V[:, ik, 0:65], eAB2_p[:, 2 * j + 0, :],
                        start=first, stop=last,
                    )
                    nc.tensor.matmul(
                        otB[0:65, :], V[:, ik, 65:130], eAB2_p[:, 2 * j + 1, :],
                        start=first, stop=last,
                    )
                if base + 2 >= nch:  # last chunks of this (pair, jq)
                    emit_norm(pair, jq, otA, otB)

            pend = []  # global software pipeline: scores/exp run 1 ahead of PV
            for pair in range(NPAIR):
                for jq in range(NJQ):
                    qw = slice(jq * 512, (jq + 1) * 512)
                    otA = otp.tile([P, 512], f32, tag="otA", name="otA", bufs=2)
                    otB = otp.tile([P, 512], f32, tag="otB", name="otB", bufs=2)
                    nch = 4 * jq + 4
                    for sc in range(nch // 2):
                        sAB2 = scp.tile([P, 4, 512], f32, tag="sAB", name="sAB", bufs=1)
                        for j in (0, 1):
                            ik = 2 * sc + j
                            kt_ = slice(ik * P, (ik + 1) * P)
                            nc.tensor.matmul(
                                sAB2[:, 2 * j + 0, :], KT[0:64, kt_],
                                QT[pair][0:64, qw], start=True, stop=True,
                            )
                            nc.tensor.matmul(
                                sAB2[:, 2 * j + 1, :], KT[64:128, kt_],
                                QT[pair][64:128, qw], start=True, stop=True,
                            )
                        eAB2 = ex.tile([P, 4, 512], bf16, tag="eAB", name="eAB", bufs=3)
                        nc.scalar.activation(eAB2[:], sAB2[:], EXP, scale=0.125)
                        for j in (0, 1):
                            d = 2 * sc + j - 4 * jq
                            if d >= 0:  # diagonal chunk: causal mask (both heads)
                                nc.vector.tensor_tensor(
                                    out=eAB2[:, 2 * j : 2 * j + 2, :],
                                    in0=eAB2[:, 2 * j : 2 * j + 2, :],
                                    in1=mask_sb[:, d : d + 1, :].to_broadcast(
                                        (P, 2, 512)
                                    ),
                                    op=MULT,
                                )
                        pend.append((eAB2, 2 * sc, otA, otB, nch, pair, jq))
                        if len(pend) > 1:
                            emit_pv(pend.pop(0))
            while pend:
                emit_pv(pend.pop(0))

        # ---- phase 3: wo projection (lo/hi halves overlap last AGs) --
        with tc.tile_pool(name="ops", bufs=3, space="PSUM") as ops:
            for h in range(2):
                for mm in range(8):
                    m = h * 8 + mm
                    mps = ops.tile([P, OCOLS], f32, tag="mps", name="mps", bufs=3)
                    for c2 in range(16):
                        nc.tensor.matmul(
                            mps[:],
                            chunks[c2][h][:, mm * P : (mm + 1) * P],
                            wo_sb[:, c2, :],
                            start=(c2 == 0),
                            stop=(c2 == 15),
                        )
                    osb = wos.tile([P, OCOLS], f32, tag="osb", name="osb", bufs=3)
                    nc.vector.tensor_copy(out=osb[:], in_=mps[:])
                    nc.sync.dma_start(out.ap()[m * P : (m + 1) * P, :], osb[:])


# ---------------------------------------------------------------------------
# host side
# ---------------------------------------------------------------------------

_PERM64 = np.concatenate([np.arange(0, 64, 2), np.arange(1, 64, 2)])


def _qcols(r):
    cols = []
    for t in range(NREP):
        for half in range(2):
            h = (2 * r + half) * NREP + t
            cols.extend(64 * h + _PERM64)
    return np.array(cols)


def _kcols(r):
    cols = []
    for half in range(2):
        g = 2 * r + half
        cols.extend(64 * g + _PERM64)
    return np.array(cols)


def _worows():
    rows = []
    for t in range(NREP):
        for rr in range(GRP):
            for half in range(2):
                h = (2 * rr + half) * NREP + t
                rows.extend(64 * h + np.arange(64))
    return np.array(rows)


def make_in_maps(x, wq, wk, wv, wo, freqs_cos, freqs_sin):
    cosT = np.ascontiguousarray(freqs_cos.T).astype(np.float32)  # (32, S)
    sinT = np.ascontiguousarray(freqs_sin.T).astype(np.float32)
    cos4 = np.ascontiguousarray(np.tile(cosT, (4, 1)))           # (128, S)
    sin4 = np.ascontiguousarray(
        np.concatenate([-sinT, sinT, -sinT, sinT], axis=0)
    )
    cmask = np.triu(np.ones((512, 512), dtype=np.float32)).astype(_BF)

    xT = [np.ascontiguousarray(x[b].T).astype(_BF) for b in range(B)]
    wo_perm = wo[_worows(), :]

    in_maps = []
    for c in range(NCORES):
        b, r = c // GRP, c % GRP
        in_maps.append(
            {
                "xT": xT[b],
                "wq": np.ascontiguousarray(wq[:, _qcols(r)]).astype(_BF),
                "wk": np.ascontiguousarray(wk[:, _kcols(r)]).astype(_BF),
                "wv": np.ascontiguousarray(wv[:, 128 * r : 128 * (r + 1)]).astype(_BF),
                "wo": np.ascontiguousarray(
                    wo_perm[:, OCOLS * r : OCOLS * (r + 1)]
                ).astype(_BF),
                "cos4": cos4,
                "sin4": sin4,
                "cmask": cmask,
            }
        )
    return in_maps


_NC_CACHE = None


def _get_nc():
    global _NC_CACHE
    if _NC_CACHE is None:
        _NC_CACHE = build_graph()
    return _NC_CACHE


def kernel(x, wq, wk, wv, wo, freqs_cos, freqs_sin):
    x = np.asarray(x)
    wq = np.asarray(wq)
    wk = np.asarray(wk)
    wv = np.asarray(wv)
    wo = np.asarray(wo)
    freqs_cos = np.asarray(freqs_cos)
    freqs_sin = np.asarray(freqs_sin)

    in_maps = make_in_maps(x, wq, wk, wv, wo, freqs_cos, freqs_sin)
    nc = _get_nc()
    res = run_bass_kernel_spmd(nc, in_maps, core_ids=list(range(NCORES)))

    out = np.empty((B, S, D), dtype=np.float32)
    for c in range(NCORES):
        b, r = c // GRP, c % GRP
        out[b, :, OCOLS * r : OCOLS * (r + 1)] = res.results[c]["out"]
    return out
